# revision 1
# baseline (speedup 1.0000x reference)
"""Multi-head self-attention (B=2, C=512, H=W=64, 8 heads) on 8 Trainium2 cores.

Sharding: data-parallel over B x head-parallel (2 heads/core). Core c handles
batch b = c//4 and heads {2*(c%4), 2*(c%4)+1} -- a contiguous 128-wide slice of
the 512-dim channel space.

Everything is laid out to avoid transposes entirely:
  - x[b] viewed as [C, S] is tok^T already (S = H*W = 4096 tokens).
  - q^T, k^T computed as [d2=128, S] (both heads stacked on partitions).
  - scores are computed TRANSPOSED: scoresT[t, s] = sum_d kT[d,t] qT[d,s],
    so softmax's exp is along the free dim and attn.V contracts t on partitions.
  - No max-subtraction needed: scores/8 ~ N(0, 0.33), exp never overflows.
  - The softmax denominator is obtained by appending a ones-column to V:
    one matmul yields both attn.V and sum(exp) rows.
  - Normalization (1/denom, varies along free dim) commutes with nothing on
    the partition axis, so it is done with a GPSIMD partition-broadcast plus
    a DVE multiply.
  - Output projection is input-column sharded: each core contributes
    attn_out[:, d_slice] @ Wp[:, d_slice].T; host sums the 4 partials per b.
    The V bias contribution (bv_slice @ WpT_slice) is folded into a
    host-precomputed per-core projection bias, so V needs no on-device bias.

All matmuls run as float32r (single-pass reduced-precision fp32, ~1.5e-4 max
rel err, ~3x faster than 2-pass fp32). exp runs on the scalar (ACT) engine
(33.5M exps/core ~ 276us busy); the PE stream (scores + attn.V, ~1.15us per
128-key x 1024-query unit) is software-pipelined against it: scores(t+1) is
issued before attn.V(t) so the PE never stalls on exp and the HAM clock
throttle stays disengaged (cold-clock matmuls are ~2x slower).

Measured on 8 axon-attached trn2 cores: ~381us HW exec, overall rel err
~6.8e-5 vs the fp32 jax reference (error entirely from f32r rounding).
"""

import os
import sys

sys.path.insert(0, "/opt/trn_rl_repo")

import numpy as np

NCORES = 8
B, C, HH, WW = 2, 512, 64, 64
S = HH * WW            # 4096 tokens
NH, D = 8, 64          # heads, head dim
DSL = 128              # per-core d-slice (2 heads)
CC = C // 128          # 4 contraction chunks
TCH = S // 128         # 32 key chunks
SBLK = 1024            # queries per attention block
NSB = S // SBLK        # 4
NSC = S // 512         # 8 (512-wide matmul slices)

_cached = {}

LAST_EXEC_NS = None
LAST_RESULTS = None


def _build():
    import concourse.mybir as mybir
    import concourse.tile as tile
    from bass_rust import add_dep_helper
    from concourse import bacc

    f32 = mybir.dt.float32
    f32r = mybir.dt.float32r
    AF = mybir.ActivationFunctionType

    nc = bacc.Bacc("TRN2", target_bir_lowering=False, debug=False,
                   num_devices=NCORES)

    xb = nc.dram_tensor("xb", [C, S], f32r, kind="ExternalInput")
    wq = nc.dram_tensor("wq", [128, CC, 128], f32r, kind="ExternalInput")
    wk = nc.dram_tensor("wk", [128, CC, 128], f32r, kind="ExternalInput")
    wv = nc.dram_tensor("wv", [128, CC, 128], f32r, kind="ExternalInput")
    wp = nc.dram_tensor("wp", [128, CC, 128], f32r, kind="ExternalInput")
    bqk = nc.dram_tensor("bqk", [128, 2], f32, kind="ExternalInput")
    pbias = nc.dram_tensor("pbias", [128, CC], f32, kind="ExternalInput")
    o = nc.dram_tensor("o", [C, S], f32, kind="ExternalOutput")

    with tile.TileContext(nc) as tc:
        with (
            tc.tile_pool(name="weights", bufs=1) as wpool,
            tc.tile_pool(name="tok", bufs=1) as tokpool,
            tc.tile_pool(name="qkv", bufs=1) as qkvpool,
            tc.tile_pool(name="exps", bufs=4) as exppool,
            tc.tile_pool(name="norm", bufs=4) as normpool,
            tc.tile_pool(name="outp", bufs=3) as outpool,
        ):
            wq_sb = wpool.tile([128, CC, 128], f32r, name="wq_sb")
            nc.sync.dma_start(out=wq_sb[:], in_=wq.ap())
            wk_sb = wpool.tile([128, CC, 128], f32r, name="wk_sb")
            nc.sync.dma_start(out=wk_sb[:], in_=wk.ap())
            wv_sb = wpool.tile([128, CC, 128], f32r, name="wv_sb")
            nc.sync.dma_start(out=wv_sb[:], in_=wv.ap())
            wp_sb = wpool.tile([128, CC, 128], f32r, name="wp_sb")
            nc.sync.dma_start(out=wp_sb[:], in_=wp.ap())
            bqk_sb = wpool.tile([128, 2], f32, name="bqk_sb")
            nc.sync.dma_start(out=bqk_sb[:], in_=bqk.ap())
            pb_sb = wpool.tile([128, CC], f32, name="pb_sb")
            nc.sync.dma_start(out=pb_sb[:], in_=pbias.ap())

            # tok^T in [partition, c_chunk, s] layout; DMA rearranges rows.
            tok_sb = tokpool.tile([128, CC, S], f32r, name="tok_sb")
            x_re = xb.ap().rearrange("(cc p) s -> p cc s", p=128)
            for qtr in range(4):
                for cc in range(CC):
                    for hf in range(2):
                        a = qtr * (S // 4) + hf * (S // 8)
                        sl = slice(a, a + S // 8)
                        nc.sync.dma_start(out=tok_sb[:, cc, sl],
                                          in_=x_re[:, cc, sl])

            qT2 = qkvpool.tile([128, S], f32r, name="qT2")
            # k^T is stored twice, zero-padded to a full K=128 contraction:
            # kTp0 = [kT_pair0; 0], kTp1 = [0; kT_pair1]. A K=128 f32r matmul
            # streams 2x faster than K=64 (measured 336 vs 526 ns), and the
            # zero rows nullify the other pair's rows of the shared qT2.
            kTp0 = qkvpool.tile([128, S], f32r, name="kTp0")
            kTp1 = qkvpool.tile([128, S], f32r, name="kTp1")
            zer32 = qkvpool.tile([64, 512], f32, name="zer32")
            nc.vector.memset(zer32[:], 0.0)
            nc.vector.tensor_copy(kTp0[64:128, 0:512], zer32[:])
            for j in range(1, 8):
                nc.vector.tensor_copy(kTp0[64:128, j * 512:(j + 1) * 512], zer32[:])
            for j in range(8):
                nc.vector.tensor_copy(kTp1[0:64, j * 512:(j + 1) * 512], zer32[:])
            # v with a ones column per key-chunk, per pair: [t, chunk, 65]
            v1_0 = qkvpool.tile([128, TCH, 65], f32r, name="v1_0")
            v1_1 = qkvpool.tile([128, TCH, 65], f32r, name="v1_1")
            ones32 = qkvpool.tile([128, TCH], f32, name="ones32")
            nc.vector.memset(ones32[:], 1.0)
            nc.vector.tensor_copy(v1_0[:, :, 64], ones32[:])
            nc.vector.tensor_copy(v1_1[:, :, 64], ones32[:])

            outT2 = qkvpool.tile([128, S], f32r, name="outT2")

            # ---- fused Q/K/V prologue, quarter-major so compute chases
            # the x DMA. V is computed transposed (efficient N=512 matmuls)
            # and flipped into [t, d] layout with PE transposes.
            ident = qkvpool.tile([128, 128], f32, name="ident")
            from concourse.masks import make_identity
            make_identity(nc, ident[:])
            ctx_psav = tc.tile_pool(name="psav", bufs=1, space="PSUM")
            pavp = ctx_psav.__enter__()
            psavs = {}
            exp_state = {"emitted": 0, "prev": None, "early": []}
            with (
                tc.tile_pool(name="psqk", bufs=2, space="PSUM") as pqkp,
                tc.tile_pool(name="pst", bufs=2, space="PSUM") as pstp,
                tc.tile_pool(name="pssce", bufs=1, space="PSUM") as pscep,
                tc.tile_pool(name="vt", bufs=2) as vtpool,
            ):
                units = [(sb, pair, tch)
                         for sb in range(NSB) for pair in range(2)
                         for tch in range(TCH)]

                def early_scores(u):
                    sb, pair, tch = u
                    kTp = kTp0 if pair == 0 else kTp1
                    s0, t0 = sb * SBLK, tch * 128
                    pssc = pscep.tile([128, SBLK], f32, name="pssce")
                    for nn in range(SBLK // 512):
                        nc.tensor.matmul(
                            pssc[:, nn * 512:(nn + 1) * 512],
                            kTp[:, t0:t0 + 128],
                            qT2[:, s0 + nn * 512:s0 + (nn + 1) * 512],
                            start=True, stop=True,
                        )
                    expT = exppool.tile([128, SBLK], f32r, name="expT")
                    nc.scalar.activation(expT[:], pssc[:], AF.Exp, scale=0.125)
                    return expT

                def early_av(u, expT):
                    sb, pair, tch = u
                    v1 = v1_0 if pair == 0 else v1_1
                    if tch == 0:
                        psavs[(sb, pair)] = pavp.tile([65, SBLK], f32,
                                                      name="psav")
                    psav = psavs[(sb, pair)]
                    for nn in range(SBLK // 512):
                        nc.tensor.matmul(
                            psav[:, nn * 512:(nn + 1) * 512],
                            v1[:, tch, :],
                            expT[:, nn * 512:(nn + 1) * 512],
                            start=(tch == 0), stop=(tch == TCH - 1),
                        )

                def early_advance(k):
                    st = exp_state
                    while st["emitted"] < k:
                        i = st["emitted"]
                        cur = early_scores(units[i])
                        if i > 0:
                            early_av(units[i - 1], st["prev"])
                        st["prev"] = cur
                        st["emitted"] = i + 1

                for qtr in range(4):
                    for which in range(3):
                        w_sb = (wq_sb, wk_sb, wv_sb)[which]
                        for scq in range(2):
                            sc = qtr * 2 + scq
                            s0 = sc * 512
                            psqk = pqkp.tile([128, 512], f32, name="psqk")
                            for cc in range(CC):
                                nc.tensor.matmul(
                                    psqk[:],
                                    w_sb[:, cc, :],
                                    tok_sb[:, cc, s0:s0 + 512],
                                    start=(cc == 0), stop=(cc == CC - 1),
                                )
                            if which == 0:
                                nc.vector.tensor_scalar_add(
                                    qT2[:, s0:s0 + 512], psqk[:], bqk_sb[:, 0:1]
                                )
                            elif which == 1:
                                nc.vector.tensor_scalar_add(
                                    kTp0[0:64, s0:s0 + 512], psqk[0:64, :],
                                    bqk_sb[0:64, 1:2]
                                )
                                nc.vector.tensor_scalar_add(
                                    kTp1[64:128, s0:s0 + 512], psqk[64:128, :],
                                    bqk_sb[64:128, 1:2]
                                )
                            else:
                                vt = vtpool.tile([128, 512], f32r, name="vt")
                                nc.vector.tensor_copy(vt[:], psqk[:])
                                for tt in range(4):
                                    tch = sc * 4 + tt
                                    pst = pstp.tile([128, 128], f32, name="pst")
                                    nc.tensor.transpose(
                                        pst[:],
                                        vt[:, tt * 128:(tt + 1) * 128].bitcast(f32),
                                        ident[:],
                                    )
                                    nc.vector.tensor_copy(
                                        v1_0[:, tch, 0:64], pst[:, 0:64]
                                    )
                                    nc.vector.tensor_copy(
                                        v1_1[:, tch, 0:64], pst[:, 64:128]
                                    )
                    if qtr < 3:
                        early_advance(8 * (qtr + 1))

            # ---- attention (flash-style, no max pass), with the output
            # projection interleaved per s-block so its matmuls and output
            # DMA hide under the ACT-bound attention stream ----------------
            with (
                tc.tile_pool(name="pssc", bufs=2, space="PSUM") as pscp,
                tc.tile_pool(name="pspr", bufs=2, space="PSUM") as pprp,
                tc.tile_pool(name="avs", bufs=2) as avpool,
            ):
                pending_proj = []
                last_av = [None]

                def emit_scores(u):
                    sb, pair, tch = u
                    kTp = kTp0 if pair == 0 else kTp1
                    s0, t0 = sb * SBLK, tch * 128
                    pssc = pscp.tile([128, SBLK], f32, name="pssc")
                    for nn in range(SBLK // 512):
                        nc.tensor.matmul(
                            pssc[:, nn * 512:(nn + 1) * 512],
                            kTp[:, t0:t0 + 128],
                            qT2[:, s0 + nn * 512:s0 + (nn + 1) * 512],
                            start=True, stop=True,
                        )
                    expT = exppool.tile([128, SBLK], f32r, name="expT")
                    nc.scalar.activation(expT[:], pssc[:], AF.Exp, scale=0.125)
                    return expT

                def emit_proj(sb, gate):
                    for scn in range(SBLK // 512):
                        s0 = sb * SBLK + scn * 512
                        for m in range(CC):
                            pspr = pprp.tile([128, 512], f32, name="pspr")
                            mm = nc.tensor.matmul(
                                pspr[:], wp_sb[:, m, :], outT2[:, s0:s0 + 512],
                                start=True, stop=True,
                            )
                            if gate is not None:
                                # Keep proj behind the attention stream so the
                                # norm chain (recip etc.) finishes off-PE first.
                                add_dep_helper(mm.ins, gate.ins, sync=False,
                                               reason="defer proj past boundary")
                            po = outpool.tile([128, 512], f32, name="po")
                            nc.vector.tensor_scalar_add(
                                po[:], pspr[:], pb_sb[:, m:m + 1]
                            )
                            nc.sync.dma_start(
                                out=o.ap()[m * 128:(m + 1) * 128, s0:s0 + 512],
                                in_=po[:],
                            )

                def emit_av(u, expT):
                    sb, pair, tch = u
                    p0 = pair * 64
                    v1 = v1_0 if pair == 0 else v1_1
                    if tch == 0:
                        psavs[(sb, pair)] = pavp.tile([65, SBLK], f32,
                                                      name="psav")
                    psav = psavs[(sb, pair)]
                    for nn in range(SBLK // 512):
                        last_av[0] = nc.tensor.matmul(
                            psav[:, nn * 512:(nn + 1) * 512],
                            v1[:, tch, :],
                            expT[:, nn * 512:(nn + 1) * 512],
                            start=(tch == 0), stop=(tch == TCH - 1),
                        )
                    if tch == TCH - 1:
                        # Move to SBUF immediately (frees the PSUM bank for
                        # the next s-block), then normalize from SBUF. Done
                        # in 512-wide halves so on the final s-block the
                        # reciprocal pipeline overlaps the projection.
                        avs = avpool.tile([65, SBLK], f32, name="avs")
                        nc.vector.tensor_copy(avs[:], psav[:])
                        nh = 4 if (sb == NSB - 1 and pair == 1) else 2
                        for hh in range(nh):
                            h0 = hh * (SBLK // nh)
                            h1 = h0 + SBLK // nh
                            recip = normpool.tile([1, SBLK // 2], f32,
                                                  name="recip")
                            nc.vector.reciprocal(recip[:, :h1 - h0],
                                                 avs[64:65, h0:h1])
                            rb = normpool.tile([64, SBLK // 2], f32, name="rb")
                            nc.gpsimd.partition_broadcast(rb[:, :h1 - h0],
                                                          recip[:, :h1 - h0])
                            nc.vector.tensor_mul(
                                outT2[p0:p0 + 64,
                                      sb * SBLK + h0:sb * SBLK + h1],
                                avs[0:64, h0:h1], rb[:, :h1 - h0]
                            )
                        if pair == 1:
                            # Delay the projection a few units so the norm
                            # chain (copy/recip/broadcast/mul on DVE+GPSIMD)
                            # finishes before the in-order PE reaches the
                            # proj matmuls.
                            pending_proj.append(sb)

                start_i = exp_state["emitted"]
                prev = exp_state["prev"]
                if start_i == 0:
                    prev = emit_scores(units[0])
                    start_i = 1
                for i in range(start_i, len(units)):
                    cur = emit_scores(units[i])
                    emit_av(units[i - 1], prev)
                    prev = cur
                    if pending_proj and (i % TCH) == 16:
                        emit_proj(pending_proj.pop(0), last_av[0])
                emit_av(units[-1], prev)
                for sb in pending_proj:
                    emit_proj(sb, None)
            ctx_psav.__exit__(None, None, None)

    nc.compile()
    return nc


def _prep_core_inputs(c, x, Wq, bq, Wk, bk, Wv, bv, Wp, bp):
    b = c // 4
    hs = 128 * (c % 4)

    def wslice_T(W):
        # W[hs:hs+128, :].T rearranged to [p, cc, d]
        return np.ascontiguousarray(
            W[hs:hs + 128, :].T.reshape(CC, 128, 128).transpose(1, 0, 2)
        )

    wp_arr = np.ascontiguousarray(
        Wp[:, hs:hs + 128].reshape(CC, 128, 128).transpose(2, 0, 1)
    )
    bqk_arr = np.ascontiguousarray(
        np.stack([bq[hs:hs + 128], bk[hs:hs + 128]], axis=1)
    ).astype(np.float32)
    vec = (bv[hs:hs + 128].astype(np.float64)
           @ Wp[:, hs:hs + 128].T.astype(np.float64)) + bp.astype(np.float64) / 4.0
    pbias_arr = np.ascontiguousarray(vec.reshape(CC, 128).T).astype(np.float32)

    return {
        "xb": np.ascontiguousarray(x[b].reshape(C, S)),
        "wq": wslice_T(Wq),
        "wk": wslice_T(Wk),
        "wv": wslice_T(Wv),
        "wp": wp_arr,
        "bqk": bqk_arr,
        "pbias": pbias_arr,
    }


def kernel(x, Wq, bq, Wk, bk, Wv, bv, Wp, bp):
    global LAST_EXEC_NS, LAST_RESULTS
    from concourse.bass_utils import run_bass_kernel_spmd

    x, Wq, bq, Wk, bk, Wv, bv, Wp, bp = (
        np.asarray(a, dtype=np.float32)
        for a in (x, Wq, bq, Wk, bk, Wv, bv, Wp, bp)
    )

    if "nc" not in _cached:
        _cached["nc"] = _build()
    nc = _cached["nc"]

    in_maps = [
        _prep_core_inputs(c, x, Wq, bq, Wk, bk, Wv, bv, Wp, bp)
        for c in range(NCORES)
    ]
    trace = bool(os.environ.get("BASS_TRACE"))
    res = run_bass_kernel_spmd(nc, in_maps, core_ids=list(range(NCORES)),
                               trace=trace)
    LAST_RESULTS = res
    LAST_EXEC_NS = res.exec_time_ns

    out = np.zeros((B, C, S), dtype=np.float32)
    for c in range(NCORES):
        out[c // 4] += res.results[c]["o"]
    return out.reshape(B, C, HH, WW)



# revision 24
# speedup vs baseline: 1.0066x; 1.0066x over previous
"""Multi-head self-attention (B=2, C=512, H=W=64, 8 heads) on 8 Trainium2 cores.

Sharding: data-parallel over B x head-parallel (2 heads/core). Core c handles
batch b = c//4 and heads {2*(c%4), 2*(c%4)+1} -- a contiguous 128-wide slice of
the 512-dim channel space.

Structure (v2 -- fp8 DoubleRow AV + ACT/DVE exp split):
  - x[b] viewed as [C, S] is tok^T already (S = H*W = 4096 tokens).
  - q^T computed as [d2=128, S] f32r with bias; k^T stored zero-padded per
    head pair (kTp0=[k0;0], kTp1=[0;k1]) WITHOUT bias: the k-bias terms are
    query-only in the scores and cancel in softmax.
  - scores computed transposed in [key, query] tiles of [128, 2, 512]
    (two 128-key chunks x 512 queries) in PSUM, f32r matmuls K=128.
  - exp: scores/8 ~ N(0, 0.33) so no max-subtraction. Split across engines:
      * ACT: exp activation (scale=1/8) writing float8e4 into SBUF.
      * DVE (a tunable subset of units): Schraudolph-style magic exp --
        uint8 = trunc(score*log2(e) + 56.15) bitcast as float8e4 gives
        2^(score*log2e - 7-ish) = exp(score/8)*2^k with ~4% rms error (the
        uniform scale factor 2^k cancels in softmax). One tensor_scalar op.
  - attn.V: fp8e4 DoubleRow matmuls (two 128-key chunks per pass, 0.5
    cycles/row = 4x f32r): stationary v8[128, 2, 65] (ones column fused for
    the softmax denominator), moving expT[128, 2, 512], out psav[65, 512].
  - normalization: reciprocal of the denominator row + gpsimd partition
    broadcast + DVE multiply, reading psav straight from PSUM.
  - projection: input-column sharded as before; proj PSUM copied to SBUF and
    DMA'd out. ALL bias terms of the projection (bp and the bv@Wp^T from V's
    folded bias) are added on the host during the partial-sum gather.

Error: fp8 quantization of exp/V contributes ~0.7% relative error; magic-exp
on the DVE subset ~0.3%; total ~0.8% against the 2e-2 gate (norm-relative,
diluted by the bias-dominated output norm).
"""

import os
import sys

sys.path.insert(0, "/opt/trn_rl_repo")

import numpy as np

NCORES = 8
B, C, HH, WW = 2, 512, 64, 64
S = HH * WW            # 4096 tokens
NH, D = 8, 64          # heads, head dim
DSL = 128              # per-core d-slice (2 heads)
CC = C // 128          # 4 contraction chunks
TCH = S // 128         # 32 key chunks
T2 = TCH // 2          # 16 key chunk-pairs
SBLK = 512             # queries per attention strip
NSB = S // SBLK        # 8

LOG2E = float(1.4426950408889634)
MAGIC_C = float(os.environ.get("MAGIC_C", "56.15"))
# t2 indices (per 16-chunk-pair group) whose exp runs on DVE via magic trick
DVE_T2 = frozenset(
    int(t) for t in os.environ.get("DVE_T2", "1,4,6,9,11,13,15").split(",") if t != ""
)
AV_LAG = int(os.environ.get("AV_LAG", "4"))
# scores via fp8 DoubleRow: stationary (k8, k8), moving (q8, q_residual8) --
# 2x PE rate on the biggest matmul stream, ~0.1% added output error
FP8_SCORES = int(os.environ.get("FP8_SCORES", "1"))

_cached = {}

LAST_EXEC_NS = None
LAST_RESULTS = None


def _build():
    import concourse.mybir as mybir
    import concourse.tile as tile
    from bass_rust import add_dep_helper
    from concourse import bacc
    from concourse.masks import make_identity

    f32 = mybir.dt.float32
    f32r = mybir.dt.float32r
    f8 = mybir.dt.float8e4
    u8 = mybir.dt.uint8
    AF = mybir.ActivationFunctionType
    DR = mybir.MatmulPerfMode.DoubleRow

    nc = bacc.Bacc("TRN2", target_bir_lowering=False, debug=False,
                   num_devices=NCORES)

    xb = nc.dram_tensor("xb", [C, S], f32r, kind="ExternalInput")
    wq = nc.dram_tensor("wq", [128, CC, 128], f32r, kind="ExternalInput")
    wk = nc.dram_tensor("wk", [128, CC, 128], f32r, kind="ExternalInput")
    wv = nc.dram_tensor("wv", [128, CC, 128], f32r, kind="ExternalInput")
    wp = nc.dram_tensor("wp", [128, CC, 128], f32r, kind="ExternalInput")
    bq = nc.dram_tensor("bq", [128, 1], f32, kind="ExternalInput")
    o = nc.dram_tensor("o", [C, S], f32, kind="ExternalOutput")

    with tile.TileContext(nc) as tc:
        with (
            tc.tile_pool(name="weights", bufs=1) as wpool,
            tc.tile_pool(name="tok", bufs=1) as tokpool,
            tc.tile_pool(name="qkv", bufs=1) as qkvpool,
            tc.tile_pool(name="exps", bufs=6) as exppool,
            tc.tile_pool(name="norm", bufs=4) as normpool,
            tc.tile_pool(name="outp", bufs=3) as outpool,
        ):
            wq_sb = wpool.tile([128, CC, 128], f32r, name="wq_sb")
            nc.sync.dma_start(out=wq_sb[:], in_=wq.ap())
            wk_sb = wpool.tile([128, CC, 128], f32r, name="wk_sb")
            nc.sync.dma_start(out=wk_sb[:], in_=wk.ap())
            wv_sb = wpool.tile([128, CC, 128], f32r, name="wv_sb")
            nc.sync.dma_start(out=wv_sb[:], in_=wv.ap())
            wp_sb = wpool.tile([128, CC, 128], f32r, name="wp_sb")
            nc.sync.dma_start(out=wp_sb[:], in_=wp.ap())
            bq_sb = wpool.tile([128, 1], f32, name="bq_sb")
            nc.sync.dma_start(out=bq_sb[:], in_=bq.ap())

            # tok^T in [partition, c_chunk, s] layout; DMA rearranges rows.
            tok_sb = tokpool.tile([128, CC, S], f32r, name="tok_sb")
            x_re = xb.ap().rearrange("(cc p) s -> p cc s", p=128)
            for qtr in range(4):
                for cc in range(CC):
                    for hf in range(2):
                        a = qtr * (S // 4) + hf * (S // 8)
                        sl = slice(a, a + S // 8)
                        nc.sync.dma_start(out=tok_sb[:, cc, sl],
                                          in_=x_re[:, cc, sl])

            zer32 = qkvpool.tile([64, 512], f32, name="zer32")
            nc.vector.memset(zer32[:], 0.0)
            if FP8_SCORES:
                # q8x planes: 0 = fp8(q), 1 = fp8(q - fp8(q)) (residual).
                # k8dP[:, tch, plane, :]: the chunk's fp8 k duplicated in
                # both planes (plane 1 filled by SBUF->SBUF DMA); pair
                # padding rows zeroed once via DMA from the zeros tile.
                q8x = qkvpool.tile([128, 2, S], f8, name="q8x")
                k8d0 = qkvpool.tile([128, TCH, 2, 128], f8, name="k8d0")
                k8d1 = qkvpool.tile([128, TCH, 2, 128], f8, name="k8d1")
                z8 = zer32[:].bitcast(f8)  # [64, 2048] of zero bytes
                for c8 in range(TCH // 8):
                    nc.sync.dma_start(
                        out=k8d0[64:128, c8 * 8:(c8 + 1) * 8, :, :].rearrange(
                            "p a b d -> p (a b d)"),
                        in_=z8,
                    )
                    nc.sync.dma_start(
                        out=k8d1[0:64, c8 * 8:(c8 + 1) * 8, :, :].rearrange(
                            "p a b d -> p (a b d)"),
                        in_=z8,
                    )
                qT2 = kTp0 = kTp1 = None
            else:
                # k^T stored twice, zero-padded to a full K=128 contraction:
                # kTp0 = [kT_pair0; 0], kTp1 = [0; kT_pair1]. The zero rows
                # nullify the other pair's rows of the shared qT2.
                qT2 = qkvpool.tile([128, S], f32r, name="qT2")
                kTp0 = qkvpool.tile([128, S], f32r, name="kTp0")
                kTp1 = qkvpool.tile([128, S], f32r, name="kTp1")
                for j in range(8):
                    nc.vector.tensor_copy(
                        kTp0[64:128, j * 512:(j + 1) * 512], zer32[:])
                    nc.vector.tensor_copy(
                        kTp1[0:64, j * 512:(j + 1) * 512], zer32[:])
            # v in fp8, chunk-pair layout for DoubleRow: [t, t2, j, 65] with
            # a fused ones column (65th) per chunk for the denominator.
            # stationary free (2*MV) must be a multiple of 64 on TRN2
            # DoubleRow (M=65 fails the ISA check): v in cols 0-63, the
            # denominator ones column at 64, zero padding 65-95.
            MV = 96
            v8_0 = qkvpool.tile([128, T2, 2, MV], f8, name="v8_0")
            v8_1 = qkvpool.tile([128, T2, 2, MV], f8, name="v8_1")
            ones32 = qkvpool.tile([128, T2, 2], f32, name="ones32")
            nc.vector.memset(ones32[:], 1.0)
            nc.vector.tensor_copy(v8_0[:, :, :, 64], ones32[:])
            nc.vector.tensor_copy(v8_1[:, :, :, 64], ones32[:])
            zpad = qkvpool.tile([128, T2, 2, MV - 65], f32, name="zpad")
            nc.vector.memset(zpad[:], 0.0)
            nc.vector.tensor_copy(v8_0[:, :, :, 65:MV], zpad[:])
            nc.vector.tensor_copy(v8_1[:, :, :, 65:MV], zpad[:])

            outT2 = qkvpool.tile([128, S], f32r, name="outT2")

            ident = qkvpool.tile([128, 128], f32, name="ident")
            make_identity(nc, ident[:])

            ctx_psav = tc.tile_pool(name="psav", bufs=2, space="PSUM")
            pavp = ctx_psav.__enter__()
            psavs = {}
            exp_state = {"emitted": 0, "av_done": 0, "pending": []}

            # ---- unit schedule ------------------------------------------
            # A unit is (sb, pair, t2): one head (pair), one 512-query strip
            # (sb), one 256-key chunk-pair (t2). AV accumulates over t2 in
            # order within each (sb, pair) group. The first two groups are
            # front-loaded during the QKV prologue (chunk availability grows
            # with the token quarters), at most 2 groups in flight (2 psav
            # PSUM buffers).
            units = []
            units += [(0, 0, t) for t in range(4)]                    # qtr 0
            units += [(0, 1, t) for t in range(4)]
            units += [(0, 0, t) for t in range(4, 8)]                 # qtr 1
            units += [(0, 1, t) for t in range(4, 8)]
            units += [(0, 0, t) for t in range(8, 12)]                # qtr 2
            units += [(0, 1, t) for t in range(8, 12)]
            units += [(0, 0, t) for t in range(12, 16)]               # main
            units += [(0, 1, t) for t in range(12, 16)]
            for sb in range(1, NSB):
                for pair in range(2):
                    units += [(sb, pair, t) for t in range(T2)]
            # scores+exp advance right after the quarter's K lands; AV
            # trails after the quarter's V (it needs v8, and the PE is
            # in-order so AV may not be emitted ahead of the transposes).
            N_EARLY_SC = [8, 16, 24]
            N_EARLY_AV = [6, 14, 22]

            def is_dve_unit(i):
                if i < 20:
                    return False  # prologue units: DVE is busy with QKV
                return units[i][2] in DVE_T2

            def emit_exp(pssc_ap, expT, dve):
                if dve:
                    nc.vector.tensor_scalar(
                        expT[:], pssc_ap, LOG2E, MAGIC_C,
                        mybir.AluOpType.mult, mybir.AluOpType.add,
                    )
                else:
                    nc.scalar.activation(expT[:].bitcast(f8), pssc_ap,
                                         AF.Exp, scale=0.125)

            def emit_av(u, expT):
                sb, pair, t2 = u
                v8 = v8_0 if pair == 0 else v8_1
                if t2 == 0:
                    psavs[(sb, pair)] = pavp.tile([MV, SBLK], f32,
                                                  name="psav")
                psav = psavs[(sb, pair)]
                return nc.tensor.matmul(
                    psav[:],
                    v8[:, t2, :, :],
                    expT[:].bitcast(f8),
                    start=(t2 == 0), stop=(t2 == T2 - 1),
                    perf_mode=DR,
                )

            def emit_norm(u):
                sb, pair, t2 = u
                p0 = pair * 64
                psav = psavs.pop((sb, pair))
                recip = normpool.tile([1, SBLK], f32, name="recip")
                nc.vector.reciprocal(recip[:], psav[64:65, :])
                rb = normpool.tile([64, SBLK], f32, name="rb")
                nc.gpsimd.partition_broadcast(rb[:], recip[:])
                nc.vector.tensor_mul(
                    outT2[p0:p0 + 64, sb * SBLK:(sb + 1) * SBLK],
                    psav[0:64, :], rb[:],
                )

            # ---- fused Q/K/V prologue, quarter-major so compute chases
            # the x DMA. V is computed transposed (efficient N=512 matmuls)
            # and flipped into [t, d] fp8 layout with PE transposes.
            with (
                tc.tile_pool(name="psqk", bufs=3, space="PSUM") as pqkp,
                tc.tile_pool(name="pssce", bufs=2, space="PSUM") as pscep,
                tc.tile_pool(name="vt", bufs=2) as vtpool,
            ):
                def early_scores(u):
                    sb, pair, t2 = u
                    kTp = kTp0 if pair == 0 else kTp1
                    s0 = sb * SBLK
                    expT = exppool.tile([128, 2, SBLK], u8, name="expT")
                    for j in range(2):
                        t0 = (2 * t2 + j) * 128
                        pssc = pscep.tile([128, SBLK], f32, name="pssce")
                        nc.tensor.matmul(
                            pssc[:], kTp[:, t0:t0 + 128],
                            qT2[:, s0:s0 + SBLK],
                            start=True, stop=True,
                        )
                        nc.scalar.activation(expT[:, j, :].bitcast(f8),
                                             pssc[:], AF.Exp, scale=0.125)
                    return expT

                def early_scores_advance(k):
                    st = exp_state
                    while st["emitted"] < k:
                        i = st["emitted"]
                        st["pending"].append((units[i], early_scores(units[i])))
                        st["emitted"] = i + 1

                def early_av_advance(k):
                    st = exp_state
                    while st["av_done"] < k:
                        u, expT = st["pending"].pop(0)
                        emit_av(u, expT)
                        st["av_done"] += 1

                for qtr in range(4):
                    for which in range(3):
                        w_sb = (wq_sb, wk_sb, wv_sb)[which]
                        for scq in range(2):
                            sc = qtr * 2 + scq
                            s0 = sc * 512
                            psqk = pqkp.tile([128, 512], f32, name="psqk")
                            for cc in range(CC):
                                nc.tensor.matmul(
                                    psqk[:],
                                    w_sb[:, cc, :],
                                    tok_sb[:, cc, s0:s0 + 512],
                                    start=(cc == 0), stop=(cc == CC - 1),
                                )
                            if which == 0:
                                nc.vector.tensor_scalar_add(
                                    qT2[:, s0:s0 + 512], psqk[:], bq_sb[:, 0:1]
                                )
                            elif which == 1:
                                # no k bias: its score terms are query-only
                                # and cancel in softmax
                                nc.vector.tensor_copy(
                                    kTp0[0:64, s0:s0 + 512], psqk[0:64, :]
                                )
                                nc.vector.tensor_copy(
                                    kTp1[64:128, s0:s0 + 512], psqk[64:128, :]
                                )
                            else:
                                vt = vtpool.tile([128, 512], f32r, name="vt")
                                nc.vector.tensor_copy(vt[:], psqk[:])
                                # 4 transposes into one PSUM tile, then two
                                # strided fp8 copies peel the head halves.
                                pst4 = pqkp.tile([128, 512], f32, name="psqk")
                                for tt in range(4):
                                    nc.tensor.transpose(
                                        pst4[:, tt * 128:(tt + 1) * 128],
                                        vt[:, tt * 128:(tt + 1) * 128]
                                        .bitcast(f32),
                                        ident[:],
                                    )
                                # chunks sc*4 .. sc*4+3 -> t2 = sc*2, sc*2+1
                                t2a = sc * 2
                                src0 = pst4[:].rearrange(
                                    "p (c d) -> p c d", c=4)[:, :, 0:64]
                                src1 = pst4[:].rearrange(
                                    "p (c d) -> p c d", c=4)[:, :, 64:128]
                                dst0 = v8_0[:, t2a:t2a + 2, :, 0:64]
                                dst1 = v8_1[:, t2a:t2a + 2, :, 0:64]
                                nc.vector.tensor_copy(dst0, src0)
                                nc.vector.tensor_copy(dst1, src1)
                        if qtr < 3 and which == 1:
                            early_scores_advance(N_EARLY_SC[qtr])
                    if qtr < 3:
                        early_av_advance(N_EARLY_AV[qtr])

            # ---- attention stream + interleaved projection ---------------
            with (
                tc.tile_pool(name="pssc", bufs=3, space="PSUM") as pscp,
            ):
                pending_proj = []
                last_av = [None]

                def emit_scores(u):
                    sb, pair, t2 = u
                    kTp = kTp0 if pair == 0 else kTp1
                    s0 = sb * SBLK
                    pssc = pscp.tile([128, 2, SBLK], f32, name="pssc")
                    for j in range(2):
                        t0 = (2 * t2 + j) * 128
                        nc.tensor.matmul(
                            pssc[:, j, :], kTp[:, t0:t0 + 128],
                            qT2[:, s0:s0 + SBLK],
                            start=True, stop=True,
                        )
                    return pssc

                def emit_proj(sb, gate, half):
                    # proj borrows a pssc-pool tile (its two 512-wide halves
                    # hold two m-chunks) so the scores pipeline can be 3 deep
                    # within the 8 PSUM banks.
                    s0 = sb * SBLK
                    pspr = pscp.tile([128, 2, SBLK], f32, name="pssc")
                    for mh in range(2):
                        m = half * 2 + mh
                        mm = nc.tensor.matmul(
                            pspr[:, mh, :], wp_sb[:, m, :],
                            outT2[:, s0:s0 + SBLK],
                            start=True, stop=True,
                        )
                        if gate is not None:
                            # Keep proj behind the attention stream so the
                            # norm chain (recip etc.) finishes off-PE first.
                            add_dep_helper(mm.ins, gate.ins, sync=False,
                                           reason="defer proj past boundary")
                        po = outpool.tile([128, SBLK], f32, name="po")
                        # on ACT (Copy is in every table set): DVE is the
                        # busier engine in the steady state
                        nc.scalar.copy(po[:], pspr[:, mh, :])
                        nc.sync.dma_start(
                            out=o.ap()[m * 128:(m + 1) * 128, s0:s0 + SBLK],
                            in_=po[:],
                        )

                def av_and_norm(pu, pexp):
                    last_av[0] = emit_av(pu, pexp)
                    if pu[2] == T2 - 1:
                        emit_norm(pu)
                        if pu[1] == 1:
                            pending_proj.append(pu[0])

                # AV consumption lags the scores/exp stream by AV_LAG units
                # so the in-order PE never stalls waiting for an exp that
                # just issued (exp latency + sem delay ~1.3us would otherwise
                # gate every unit).
                start_i = exp_state["emitted"]
                pending = exp_state["pending"]
                for i in range(start_i, len(units)):
                    u = units[i]
                    pssc = emit_scores(u)
                    cur = exppool.tile([128, 2, SBLK], u8, name="expT")
                    emit_exp(pssc[:], cur, is_dve_unit(i))
                    pending.append((u, cur))
                    if len(pending) > AV_LAG:
                        av_and_norm(*pending.pop(0))
                    if pending_proj and (i % T2) == 8:
                        emit_proj(pending_proj[0], last_av[0], 0)
                    elif pending_proj and (i % T2) == 12:
                        emit_proj(pending_proj.pop(0), last_av[0], 1)
                for pu, pexp in pending:
                    av_and_norm(pu, pexp)
                for sb in pending_proj:
                    emit_proj(sb, None, 0)
                    emit_proj(sb, None, 1)
            ctx_psav.__exit__(None, None, None)

    nc.compile()
    return nc


def _prep_core_inputs(c, x, Wq, bq, Wk, bk, Wv, bv, Wp, bp):
    b = c // 4
    hs = 128 * (c % 4)

    def wslice_T(W):
        # W[hs:hs+128, :].T rearranged to [p, cc, d]
        return np.ascontiguousarray(
            W[hs:hs + 128, :].T.reshape(CC, 128, 128).transpose(1, 0, 2)
        )

    wp_arr = np.ascontiguousarray(
        Wp[:, hs:hs + 128].reshape(CC, 128, 128).transpose(2, 0, 1)
    )

    return {
        "xb": np.ascontiguousarray(x[b].reshape(C, S)),
        "wq": wslice_T(Wq),
        "wk": wslice_T(Wk),
        "wv": wslice_T(Wv),
        "wp": wp_arr,
        "bq": np.ascontiguousarray(bq[hs:hs + 128, None]).astype(np.float32),
    }


def kernel(x, Wq, bq, Wk, bk, Wv, bv, Wp, bp):
    global LAST_EXEC_NS, LAST_RESULTS
    from concourse.bass_utils import run_bass_kernel_spmd

    x, Wq, bq, Wk, bk, Wv, bv, Wp, bp = (
        np.asarray(a, dtype=np.float32)
        for a in (x, Wq, bq, Wk, bk, Wv, bv, Wp, bp)
    )

    if "nc" not in _cached:
        _cached["nc"] = _build()
    nc = _cached["nc"]

    in_maps = [
        _prep_core_inputs(c, x, Wq, bq, Wk, bk, Wv, bv, Wp, bp)
        for c in range(NCORES)
    ]
    trace = bool(os.environ.get("BASS_TRACE"))
    res = run_bass_kernel_spmd(nc, in_maps, core_ids=list(range(NCORES)),
                               trace=trace)
    LAST_RESULTS = res
    LAST_EXEC_NS = res.exec_time_ns

    # The projection bias (bp) and V's bias routed through the projection
    # (bv @ Wp^T) are constant per output channel: added host-side during
    # the partial-sum gather.
    bias_total = (bv.astype(np.float64) @ Wp.T.astype(np.float64)
                  + bp.astype(np.float64)).astype(np.float32)
    out = np.zeros((B, C, S), dtype=np.float32)
    for c in range(NCORES):
        out[c // 4] += res.results[c]["o"]
    out += bias_total[None, :, None]
    return out.reshape(B, C, HH, WW)


# revision 31
# speedup vs baseline: 1.1075x; 1.1002x over previous
"""Multi-head self-attention (B=2, C=512, H=W=64, 8 heads) on 8 Trainium2 cores.

Sharding: data-parallel over B x head-parallel (2 heads/core). Core c handles
batch b = c//4 and heads {2*(c%4), 2*(c%4)+1} -- a contiguous 128-wide slice of
the 512-dim channel space.

Structure (v2 -- fp8 DoubleRow AV + ACT/DVE exp split):
  - x[b] viewed as [C, S] is tok^T already (S = H*W = 4096 tokens).
  - q^T computed as [d2=128, S] f32r with bias; k^T stored zero-padded per
    head pair (kTp0=[k0;0], kTp1=[0;k1]) WITHOUT bias: the k-bias terms are
    query-only in the scores and cancel in softmax.
  - scores computed transposed in [key, query] tiles of [128, 2, 512]
    (two 128-key chunks x 512 queries) in PSUM, f32r matmuls K=128.
  - exp: scores/8 ~ N(0, 0.33) so no max-subtraction. Split across engines:
      * ACT: exp activation (scale=1/8) writing float8e4 into SBUF.
      * DVE (a tunable subset of units): Schraudolph-style magic exp --
        uint8 = trunc(score*log2(e) + 56.15) bitcast as float8e4 gives
        2^(score*log2e - 7-ish) = exp(score/8)*2^k with ~4% rms error (the
        uniform scale factor 2^k cancels in softmax). One tensor_scalar op.
  - attn.V: fp8e4 DoubleRow matmuls (two 128-key chunks per pass, 0.5
    cycles/row = 4x f32r): stationary v8[128, 2, 65] (ones column fused for
    the softmax denominator), moving expT[128, 2, 512], out psav[65, 512].
  - normalization: reciprocal of the denominator row + gpsimd partition
    broadcast + DVE multiply, reading psav straight from PSUM.
  - projection: input-column sharded as before; proj PSUM copied to SBUF and
    DMA'd out. ALL bias terms of the projection (bp and the bv@Wp^T from V's
    folded bias) are added on the host during the partial-sum gather.

Error: fp8 quantization of exp/V contributes ~0.7% relative error; magic-exp
on the DVE subset ~0.3%; total ~0.8% against the 2e-2 gate (norm-relative,
diluted by the bias-dominated output norm).
"""

import os
import sys

sys.path.insert(0, "/opt/trn_rl_repo")

import numpy as np

NCORES = 8
B, C, HH, WW = 2, 512, 64, 64
S = HH * WW            # 4096 tokens
NH, D = 8, 64          # heads, head dim
DSL = 128              # per-core d-slice (2 heads)
CC = C // 128          # 4 contraction chunks
TCH = S // 128         # 32 key chunks
T2 = TCH // 2          # 16 key chunk-pairs
SBLK = 512             # queries per attention strip
NSB = S // SBLK        # 8

LOG2E = float(1.4426950408889634)
MAGIC_C = float(os.environ.get("MAGIC_C", "56.15"))
# t2 indices (per 16-chunk-pair group) whose exp runs on DVE via magic trick
DVE_T2 = frozenset(
    int(t) for t in os.environ.get("DVE_T2", "1,4,6,9,11,13,15").split(",") if t != ""
)
AV_LAG = int(os.environ.get("AV_LAG", "4"))


_cached = {}

LAST_EXEC_NS = None
LAST_RESULTS = None


def _build():
    import concourse.mybir as mybir
    import concourse.tile as tile
    from bass_rust import add_dep_helper
    from concourse import bacc
    from concourse.masks import make_identity

    f32 = mybir.dt.float32
    f32r = mybir.dt.float32r
    bf16 = mybir.dt.bfloat16
    f8 = mybir.dt.float8e4
    u8 = mybir.dt.uint8
    AF = mybir.ActivationFunctionType
    DR = mybir.MatmulPerfMode.DoubleRow

    nc = bacc.Bacc("TRN2", target_bir_lowering=False, debug=False,
                   num_devices=NCORES)

    xb = nc.dram_tensor("xb", [C, S], bf16, kind="ExternalInput")
    wq = nc.dram_tensor("wq", [128, CC, 128], bf16, kind="ExternalInput")
    wk = nc.dram_tensor("wk", [128, CC, 128], bf16, kind="ExternalInput")
    wv = nc.dram_tensor("wv", [128, CC, 128], bf16, kind="ExternalInput")
    wp = nc.dram_tensor("wp", [128, CC, 128], bf16, kind="ExternalInput")
    bq = nc.dram_tensor("bq", [128, 1], f32, kind="ExternalInput")
    o = nc.dram_tensor("o", [C, S], f32, kind="ExternalOutput")

    with tile.TileContext(nc) as tc:
        with (
            tc.tile_pool(name="weights", bufs=1) as wpool,
            tc.tile_pool(name="tok", bufs=1) as tokpool,
            tc.tile_pool(name="qkv", bufs=1) as qkvpool,
            tc.tile_pool(name="exps", bufs=6) as exppool,
            tc.tile_pool(name="norm", bufs=10) as normpool,
            tc.tile_pool(name="outp", bufs=3) as outpool,
        ):
            wq_sb = wpool.tile([128, CC, 128], bf16, name="wq_sb")
            nc.sync.dma_start(out=wq_sb[:], in_=wq.ap())
            wk_sb = wpool.tile([128, CC, 128], bf16, name="wk_sb")
            nc.sync.dma_start(out=wk_sb[:], in_=wk.ap())
            wv_sb = wpool.tile([128, CC, 128], bf16, name="wv_sb")
            nc.sync.dma_start(out=wv_sb[:], in_=wv.ap())
            wp_sb = wpool.tile([128, CC, 128], bf16, name="wp_sb")
            nc.sync.dma_start(out=wp_sb[:], in_=wp.ap())
            bq_sb = wpool.tile([128, 1], f32, name="bq_sb")
            nc.sync.dma_start(out=bq_sb[:], in_=bq.ap())

            # tok^T in [partition, c_chunk, s] layout; DMA rearranges rows.
            tok_sb = tokpool.tile([128, CC, S], bf16, name="tok_sb")
            x_re = xb.ap().rearrange("(cc p) s -> p cc s", p=128)
            for qtr in range(4):
                for cc in range(CC):
                    for hf in range(2):
                        a = qtr * (S // 4) + hf * (S // 8)
                        sl = slice(a, a + S // 8)
                        nc.sync.dma_start(out=tok_sb[:, cc, sl],
                                          in_=x_re[:, cc, sl])

            zer32 = qkvpool.tile([64, 512], f32, name="zer32")
            nc.vector.memset(zer32[:], 0.0)
            # q/k in bf16 (matmul streams as fast as fp8 on this toolchain,
            # error negligible). k^T stored twice, zero-padded to a full
            # K=128 contraction: kTp0 = [kT_pair0; 0], kTp1 = [0; kT_pair1].
            # The zero rows nullify the other pair's rows of the shared qT2.
            qT2 = qkvpool.tile([128, S], bf16, name="qT2")
            kTp0 = qkvpool.tile([128, S], bf16, name="kTp0")
            kTp1 = qkvpool.tile([128, S], bf16, name="kTp1")
            for j in range(8):
                nc.vector.tensor_copy(
                    kTp0[64:128, j * 512:(j + 1) * 512], zer32[:])
                nc.vector.tensor_copy(
                    kTp1[0:64, j * 512:(j + 1) * 512], zer32[:])
            # v in fp8, chunk-pair layout for DoubleRow: [t, t2, j, 65] with
            # a fused ones column (65th) per chunk for the denominator.
            # stationary free (2*MV) must be a multiple of 64 on TRN2
            # DoubleRow (M=65 fails the ISA check): v in cols 0-63, the
            # denominator ones column at 64, zero padding 65-95.
            MV = 96
            v8_0 = qkvpool.tile([128, T2, 2, MV], f8, name="v8_0")
            v8_1 = qkvpool.tile([128, T2, 2, MV], f8, name="v8_1")
            ones32 = qkvpool.tile([128, T2, 2], f32, name="ones32")
            nc.vector.memset(ones32[:], 1.0)
            nc.vector.tensor_copy(v8_0[:, :, :, 64], ones32[:])
            nc.vector.tensor_copy(v8_1[:, :, :, 64], ones32[:])
            zpad = qkvpool.tile([128, T2, 2, MV - 65], f32, name="zpad")
            nc.vector.memset(zpad[:], 0.0)
            nc.vector.tensor_copy(v8_0[:, :, :, 65:MV], zpad[:])
            nc.vector.tensor_copy(v8_1[:, :, :, 65:MV], zpad[:])

            outT2 = qkvpool.tile([128, S], bf16, name="outT2")

            ident = qkvpool.tile([128, 128], f32, name="ident")
            make_identity(nc, ident[:])

            ctx_psav = tc.tile_pool(name="psav", bufs=2, space="PSUM")
            pavp = ctx_psav.__enter__()
            psavs = {}
            exp_state = {"emitted": 0, "av_done": 0, "pending": []}

            # ---- unit schedule ------------------------------------------
            # A unit is (sb, pair, t2): one head (pair), one 512-query strip
            # (sb), one 256-key chunk-pair (t2). AV accumulates over t2 in
            # order within each (sb, pair) group. The first two groups are
            # front-loaded during the QKV prologue (chunk availability grows
            # with the token quarters), at most 2 groups in flight (2 psav
            # PSUM buffers).
            units = []
            units += [(0, 0, t) for t in range(4)]                    # qtr 0
            units += [(0, 1, t) for t in range(4)]
            units += [(0, 0, t) for t in range(4, 8)]                 # qtr 1
            units += [(0, 1, t) for t in range(4, 8)]
            units += [(0, 0, t) for t in range(8, 12)]                # qtr 2
            units += [(0, 1, t) for t in range(8, 12)]
            units += [(0, 0, t) for t in range(12, 16)]               # main
            units += [(0, 1, t) for t in range(12, 16)]
            for sb in range(1, NSB):
                for pair in range(2):
                    units += [(sb, pair, t) for t in range(T2)]
            # scores+exp advance right after the quarter's K lands; AV
            # trails after the quarter's V (it needs v8, and the PE is
            # in-order so AV may not be emitted ahead of the transposes).
            N_EARLY_SC = [8, 16, 24]
            N_EARLY_AV = [6, 14, 22]

            def is_dve_unit(i):
                if i < 20:
                    return False  # prologue units: DVE is busy with QKV
                return units[i][2] in DVE_T2

            def emit_exp(pssc_ap, expT, dve):
                if dve:
                    nc.vector.tensor_scalar(
                        expT[:], pssc_ap, LOG2E, MAGIC_C,
                        mybir.AluOpType.mult, mybir.AluOpType.add,
                    )
                else:
                    nc.scalar.activation(expT[:].bitcast(f8), pssc_ap,
                                         AF.Exp, scale=0.125)

            def emit_av(u, expT):
                sb, pair, t2 = u
                v8 = v8_0 if pair == 0 else v8_1
                if t2 == 0:
                    psavs[(sb, pair)] = pavp.tile([MV, SBLK], f32,
                                                  name="psav")
                psav = psavs[(sb, pair)]
                return nc.tensor.matmul(
                    psav[:],
                    v8[:, t2, :, :],
                    expT[:].bitcast(f8),
                    start=(t2 == 0), stop=(t2 == T2 - 1),
                    perf_mode=DR,
                )

            def emit_norm(u):
                sb, pair, t2 = u
                p0 = pair * 64
                psav = psavs.pop((sb, pair))
                # DVE reciprocal costs ~7.5 cycles/element/lane, so the
                # [1, 512] denominator row would take ~4us on one lane
                # (and reciprocal_approx_fast returns garbage on real hw).
                # Spread the row across 128 partitions with a pair of tiny
                # SBUF<->SBUF DMAs and run the reciprocal at [128, 4].
                den = normpool.tile([1, SBLK], f32, name="den")
                nc.vector.tensor_copy(den[:], psav[64:65, :])
                dsp = normpool.tile([128, SBLK // 128], f32, name="dsp")
                nc.sync.dma_start(out=dsp[:], in_=den[:])
                rsp = normpool.tile([128, SBLK // 128], f32, name="rsp")
                nc.vector.reciprocal(rsp[:], dsp[:])
                recip = normpool.tile([1, SBLK], f32, name="recip")
                nc.sync.dma_start(out=recip[:], in_=rsp[:])
                rb = normpool.tile([64, SBLK], f32, name="rb")
                nc.gpsimd.partition_broadcast(rb[:], recip[:])
                nc.vector.tensor_mul(
                    outT2[p0:p0 + 64, sb * SBLK:(sb + 1) * SBLK],
                    psav[0:64, :], rb[:],
                )

            # ---- fused Q/K/V prologue, quarter-major so compute chases
            # the x DMA. V is computed transposed (efficient N=512 matmuls)
            # and flipped into [t, d] fp8 layout with PE transposes.
            with (
                tc.tile_pool(name="psqk", bufs=3, space="PSUM") as pqkp,
                tc.tile_pool(name="pssce", bufs=2, space="PSUM") as pscep,
                tc.tile_pool(name="vt", bufs=2) as vtpool,
            ):
                def early_scores(u):
                    sb, pair, t2 = u
                    kTp = kTp0 if pair == 0 else kTp1
                    s0 = sb * SBLK
                    expT = exppool.tile([128, 2, SBLK], u8, name="expT")
                    for j in range(2):
                        t0 = (2 * t2 + j) * 128
                        pssc = pscep.tile([128, SBLK], f32, name="pssce")
                        nc.tensor.matmul(
                            pssc[:], kTp[:, t0:t0 + 128],
                            qT2[:, s0:s0 + SBLK],
                            start=True, stop=True,
                        )
                        nc.scalar.activation(expT[:, j, :].bitcast(f8),
                                             pssc[:], AF.Exp, scale=0.125)
                    return expT

                def early_scores_advance(k):
                    st = exp_state
                    while st["emitted"] < k:
                        i = st["emitted"]
                        st["pending"].append((units[i], early_scores(units[i])))
                        st["emitted"] = i + 1

                def early_av_advance(k):
                    st = exp_state
                    while st["av_done"] < k:
                        u, expT = st["pending"].pop(0)
                        emit_av(u, expT)
                        st["av_done"] += 1

                for qtr in range(4):
                    for which in range(3):
                        w_sb = (wq_sb, wk_sb, wv_sb)[which]
                        for scq in range(2):
                            sc = qtr * 2 + scq
                            s0 = sc * 512
                            psqk = pqkp.tile([128, 512], f32, name="psqk")
                            for cc in range(CC):
                                nc.tensor.matmul(
                                    psqk[:],
                                    w_sb[:, cc, :],
                                    tok_sb[:, cc, s0:s0 + 512],
                                    start=(cc == 0), stop=(cc == CC - 1),
                                )
                            if which == 0:
                                nc.vector.tensor_scalar_add(
                                    qT2[:, s0:s0 + 512], psqk[:], bq_sb[:, 0:1]
                                )
                            elif which == 1:
                                # no k bias: its score terms are query-only
                                # and cancel in softmax
                                nc.vector.tensor_copy(
                                    kTp0[0:64, s0:s0 + 512], psqk[0:64, :]
                                )
                                nc.vector.tensor_copy(
                                    kTp1[64:128, s0:s0 + 512], psqk[64:128, :]
                                )
                            else:
                                vt = vtpool.tile([128, 512], f32r, name="vt")
                                nc.vector.tensor_copy(vt[:], psqk[:])
                                # 4 transposes into one PSUM tile, then two
                                # strided fp8 copies peel the head halves.
                                pst4 = pqkp.tile([128, 512], f32, name="psqk")
                                for tt in range(4):
                                    nc.tensor.transpose(
                                        pst4[:, tt * 128:(tt + 1) * 128],
                                        vt[:, tt * 128:(tt + 1) * 128]
                                        .bitcast(f32),
                                        ident[:],
                                    )
                                # chunks sc*4 .. sc*4+3 -> t2 = sc*2, sc*2+1
                                t2a = sc * 2
                                src0 = pst4[:].rearrange(
                                    "p (c d) -> p c d", c=4)[:, :, 0:64]
                                src1 = pst4[:].rearrange(
                                    "p (c d) -> p c d", c=4)[:, :, 64:128]
                                dst0 = v8_0[:, t2a:t2a + 2, :, 0:64]
                                dst1 = v8_1[:, t2a:t2a + 2, :, 0:64]
                                nc.vector.tensor_copy(dst0, src0)
                                nc.vector.tensor_copy(dst1, src1)
                        if qtr < 3 and which == 1:
                            early_scores_advance(N_EARLY_SC[qtr])
                    if qtr < 3:
                        early_av_advance(N_EARLY_AV[qtr])

            # ---- attention stream + interleaved projection ---------------
            with (
                tc.tile_pool(name="pssc", bufs=3, space="PSUM") as pscp,
            ):
                pending_proj = []
                last_av = [None]

                def emit_scores(u):
                    sb, pair, t2 = u
                    kTp = kTp0 if pair == 0 else kTp1
                    s0 = sb * SBLK
                    pssc = pscp.tile([128, 2, SBLK], f32, name="pssc")
                    for j in range(2):
                        t0 = (2 * t2 + j) * 128
                        nc.tensor.matmul(
                            pssc[:, j, :], kTp[:, t0:t0 + 128],
                            qT2[:, s0:s0 + SBLK],
                            start=True, stop=True,
                        )
                    return pssc

                def emit_proj(sb, gate, half):
                    # proj borrows a pssc-pool tile (its two 512-wide halves
                    # hold two m-chunks) so the scores pipeline can be 3 deep
                    # within the 8 PSUM banks.
                    s0 = sb * SBLK
                    pspr = pscp.tile([128, 2, SBLK], f32, name="pssc")
                    for mh in range(2):
                        m = half * 2 + mh
                        mm = nc.tensor.matmul(
                            pspr[:, mh, :], wp_sb[:, m, :],
                            outT2[:, s0:s0 + SBLK],
                            start=True, stop=True,
                        )
                        if gate is not None:
                            # Keep proj behind the attention stream so the
                            # norm chain (recip etc.) finishes off-PE first.
                            add_dep_helper(mm.ins, gate.ins, sync=False,
                                           reason="defer proj past boundary")
                        po = outpool.tile([128, SBLK], f32, name="po")
                        # on ACT (Copy is in every table set): DVE is the
                        # busier engine in the steady state
                        nc.scalar.copy(po[:], pspr[:, mh, :])
                        nc.sync.dma_start(
                            out=o.ap()[m * 128:(m + 1) * 128, s0:s0 + SBLK],
                            in_=po[:],
                        )

                def av_and_norm(pu, pexp):
                    last_av[0] = emit_av(pu, pexp)
                    if pu[2] == T2 - 1:
                        emit_norm(pu)
                        if pu[1] == 1:
                            pending_proj.append(pu[0])

                # AV consumption lags the scores/exp stream by AV_LAG units
                # so the in-order PE never stalls waiting for an exp that
                # just issued (exp latency + sem delay ~1.3us would otherwise
                # gate every unit).
                start_i = exp_state["emitted"]
                pending = exp_state["pending"]
                for i in range(start_i, len(units)):
                    u = units[i]
                    pssc = emit_scores(u)
                    cur = exppool.tile([128, 2, SBLK], u8, name="expT")
                    emit_exp(pssc[:], cur, is_dve_unit(i))
                    pending.append((u, cur))
                    if len(pending) > AV_LAG:
                        av_and_norm(*pending.pop(0))
                    if pending_proj and (i % T2) == 8:
                        emit_proj(pending_proj[0], last_av[0], 0)
                    elif pending_proj and (i % T2) == 12:
                        emit_proj(pending_proj.pop(0), last_av[0], 1)
                for pu, pexp in pending:
                    av_and_norm(pu, pexp)
                for sb in pending_proj:
                    emit_proj(sb, None, 0)
                    emit_proj(sb, None, 1)
            ctx_psav.__exit__(None, None, None)

    nc.compile()
    return nc


def _prep_core_inputs(c, x, Wq, bq, Wk, bk, Wv, bv, Wp, bp):
    import ml_dtypes

    b = c // 4
    hs = 128 * (c % 4)
    bft = ml_dtypes.bfloat16

    def wslice_T(W):
        # W[hs:hs+128, :].T rearranged to [p, cc, d]
        return np.ascontiguousarray(
            W[hs:hs + 128, :].T.reshape(CC, 128, 128).transpose(1, 0, 2)
        ).astype(bft)

    wp_arr = np.ascontiguousarray(
        Wp[:, hs:hs + 128].reshape(CC, 128, 128).transpose(2, 0, 1)
    ).astype(bft)

    return {
        "xb": np.ascontiguousarray(x[b].reshape(C, S)).astype(bft),
        "wq": wslice_T(Wq),
        "wk": wslice_T(Wk),
        "wv": wslice_T(Wv),
        "wp": wp_arr,
        "bq": np.ascontiguousarray(bq[hs:hs + 128, None]).astype(np.float32),
    }


def kernel(x, Wq, bq, Wk, bk, Wv, bv, Wp, bp):
    global LAST_EXEC_NS, LAST_RESULTS
    from concourse.bass_utils import run_bass_kernel_spmd

    x, Wq, bq, Wk, bk, Wv, bv, Wp, bp = (
        np.asarray(a, dtype=np.float32)
        for a in (x, Wq, bq, Wk, bk, Wv, bv, Wp, bp)
    )

    if "nc" not in _cached:
        _cached["nc"] = _build()
    nc = _cached["nc"]

    in_maps = [
        _prep_core_inputs(c, x, Wq, bq, Wk, bk, Wv, bv, Wp, bp)
        for c in range(NCORES)
    ]
    trace = bool(os.environ.get("BASS_TRACE"))
    res = run_bass_kernel_spmd(nc, in_maps, core_ids=list(range(NCORES)),
                               trace=trace)
    LAST_RESULTS = res
    LAST_EXEC_NS = res.exec_time_ns

    # The projection bias (bp) and V's bias routed through the projection
    # (bv @ Wp^T) are constant per output channel: added host-side during
    # the partial-sum gather.
    bias_total = (bv.astype(np.float64) @ Wp.T.astype(np.float64)
                  + bp.astype(np.float64)).astype(np.float32)
    out = np.zeros((B, C, S), dtype=np.float32)
    for c in range(NCORES):
        out[c // 4] += res.results[c]["o"]
    out += bias_total[None, :, None]
    return out.reshape(B, C, HH, WW)


# revision 34
# speedup vs baseline: 1.3576x; 1.2258x over previous
"""Multi-head self-attention (B=2, C=512, H=W=64, 8 heads) on 8 Trainium2 cores.

Sharding: data-parallel over B x head-parallel (2 heads/core). Core c handles
batch b = c//4 and heads {2*(c%4), 2*(c%4)+1} -- a contiguous 128-wide slice of
the 512-dim channel space.

Structure (v2 -- fp8 DoubleRow AV + ACT/DVE exp split):
  - x[b] viewed as [C, S] is tok^T already (S = H*W = 4096 tokens).
  - q^T computed as [d2=128, S] f32r with bias; k^T stored zero-padded per
    head pair (kTp0=[k0;0], kTp1=[0;k1]) WITHOUT bias: the k-bias terms are
    query-only in the scores and cancel in softmax.
  - scores computed transposed in [key, query] tiles of [128, 2, 512]
    (two 128-key chunks x 512 queries) in PSUM, f32r matmuls K=128.
  - exp: scores/8 ~ N(0, 0.33) so no max-subtraction. Split across engines:
      * ACT: exp activation (scale=1/8) writing float8e4 into SBUF.
      * DVE (a tunable subset of units): Schraudolph-style magic exp --
        uint8 = trunc(score*log2(e) + 56.15) bitcast as float8e4 gives
        2^(score*log2e - 7-ish) = exp(score/8)*2^k with ~4% rms error (the
        uniform scale factor 2^k cancels in softmax). One tensor_scalar op.
  - attn.V: fp8e4 DoubleRow matmuls (two 128-key chunks per pass, 0.5
    cycles/row = 4x f32r): stationary v8[128, 2, 65] (ones column fused for
    the softmax denominator), moving expT[128, 2, 512], out psav[65, 512].
  - normalization: reciprocal of the denominator row + gpsimd partition
    broadcast + DVE multiply, reading psav straight from PSUM.
  - projection: input-column sharded as before; proj PSUM copied to SBUF and
    DMA'd out. ALL bias terms of the projection (bp and the bv@Wp^T from V's
    folded bias) are added on the host during the partial-sum gather.

Error: fp8 quantization of exp/V contributes ~0.7% relative error; magic-exp
on the DVE subset ~0.3%; total ~0.8% against the 2e-2 gate (norm-relative,
diluted by the bias-dominated output norm).
"""

import os
import sys

sys.path.insert(0, "/opt/trn_rl_repo")

import numpy as np

NCORES = 8
B, C, HH, WW = 2, 512, 64, 64
S = HH * WW            # 4096 tokens
NH, D = 8, 64          # heads, head dim
DSL = 128              # per-core d-slice (2 heads)
CC = C // 128          # 4 contraction chunks
TCH = S // 128         # 32 key chunks
T2 = TCH // 2          # 16 key chunk-pairs
SBLK = 512             # queries per attention strip
NSB = S // SBLK        # 8

LOG2E = float(1.4426950408889634)
MAGIC_C = float(os.environ.get("MAGIC_C", "56.15"))
# t2 indices (per 16-chunk-pair group) whose exp runs on DVE via magic trick
DVE_T2 = frozenset(
    int(t) for t in os.environ.get("DVE_T2", "1,4,6,9,11,13,15").split(",") if t != ""
)
AV_LAG = int(os.environ.get("AV_LAG", "4"))


_cached = {}

LAST_EXEC_NS = None
LAST_RESULTS = None


def _build():
    import concourse.mybir as mybir
    import concourse.tile as tile
    from bass_rust import add_dep_helper
    from concourse import bacc
    from concourse.masks import make_identity

    f32 = mybir.dt.float32
    f32r = mybir.dt.float32r
    bf16 = mybir.dt.bfloat16
    f8 = mybir.dt.float8e4
    u8 = mybir.dt.uint8
    AF = mybir.ActivationFunctionType
    DR = mybir.MatmulPerfMode.DoubleRow

    nc = bacc.Bacc("TRN2", target_bir_lowering=False, debug=False,
                   num_devices=NCORES)

    xb = nc.dram_tensor("xb", [C, S], bf16, kind="ExternalInput")
    wq = nc.dram_tensor("wq", [128, CC, 128], bf16, kind="ExternalInput")
    wk = nc.dram_tensor("wk", [128, CC, 128], bf16, kind="ExternalInput")
    wv = nc.dram_tensor("wv", [128, CC, 128], bf16, kind="ExternalInput")
    wp = nc.dram_tensor("wp", [128, CC, 128], bf16, kind="ExternalInput")
    bq = nc.dram_tensor("bq", [128, 1], f32, kind="ExternalInput")
    o = nc.dram_tensor("o", [C, S], f32, kind="ExternalOutput")

    with tile.TileContext(nc) as tc:
        with (
            tc.tile_pool(name="weights", bufs=1) as wpool,
            tc.tile_pool(name="tok", bufs=1) as tokpool,
            tc.tile_pool(name="qkv", bufs=1) as qkvpool,
            tc.tile_pool(name="exps", bufs=6) as exppool,
            tc.tile_pool(name="norm", bufs=10) as normpool,
            tc.tile_pool(name="outp", bufs=3) as outpool,
        ):
            wq_sb = wpool.tile([128, CC, 128], bf16, name="wq_sb")
            nc.sync.dma_start(out=wq_sb[:], in_=wq.ap())
            wk_sb = wpool.tile([128, CC, 128], bf16, name="wk_sb")
            nc.sync.dma_start(out=wk_sb[:], in_=wk.ap())
            wv_sb = wpool.tile([128, CC, 128], bf16, name="wv_sb")
            nc.sync.dma_start(out=wv_sb[:], in_=wv.ap())
            wp_sb = wpool.tile([128, CC, 128], bf16, name="wp_sb")
            nc.sync.dma_start(out=wp_sb[:], in_=wp.ap())
            bq_sb = wpool.tile([128, 1], f32, name="bq_sb")
            nc.sync.dma_start(out=bq_sb[:], in_=bq.ap())

            # tok^T in [partition, c_chunk, s] layout; DMA rearranges rows.
            tok_sb = tokpool.tile([128, CC, S], bf16, name="tok_sb")
            x_re = xb.ap().rearrange("(cc p) s -> p cc s", p=128)
            for qtr in range(4):
                for cc in range(CC):
                    for hf in range(2):
                        a = qtr * (S // 4) + hf * (S // 8)
                        sl = slice(a, a + S // 8)
                        nc.sync.dma_start(out=tok_sb[:, cc, sl],
                                          in_=x_re[:, cc, sl])

            zer32 = qkvpool.tile([64, 512], f32, name="zer32")
            nc.vector.memset(zer32[:], 0.0)
            # q/k in bf16 (matmul streams as fast as fp8 on this toolchain,
            # error negligible). k^T stored twice, zero-padded to a full
            # K=128 contraction: kTp0 = [kT_pair0; 0], kTp1 = [0; kT_pair1].
            # The zero rows nullify the other pair's rows of the shared qT2.
            qT2 = qkvpool.tile([128, S], bf16, name="qT2")
            kTp0 = qkvpool.tile([128, S], bf16, name="kTp0")
            kTp1 = qkvpool.tile([128, S], bf16, name="kTp1")
            for j in range(8):
                nc.vector.tensor_copy(
                    kTp0[64:128, j * 512:(j + 1) * 512], zer32[:])
                nc.vector.tensor_copy(
                    kTp1[0:64, j * 512:(j + 1) * 512], zer32[:])
            # v in fp8, chunk-pair layout for DoubleRow: [t, t2, j, 65] with
            # a fused ones column (65th) per chunk for the denominator.
            # stationary free (2*MV) must be a multiple of 64 on TRN2
            # DoubleRow (M=65 fails the ISA check): v in cols 0-63, the
            # denominator ones column at 64, zero padding 65-95.
            MV = 96
            v8_0 = qkvpool.tile([128, T2, 2, MV], f8, name="v8_0")
            v8_1 = qkvpool.tile([128, T2, 2, MV], f8, name="v8_1")
            ones32 = qkvpool.tile([128, T2, 2], f32, name="ones32")
            nc.vector.memset(ones32[:], 1.0)
            nc.vector.tensor_copy(v8_0[:, :, :, 64], ones32[:])
            nc.vector.tensor_copy(v8_1[:, :, :, 64], ones32[:])
            zpad = qkvpool.tile([128, T2, 2, MV - 65], f32, name="zpad")
            nc.vector.memset(zpad[:], 0.0)
            nc.vector.tensor_copy(v8_0[:, :, :, 65:MV], zpad[:])
            nc.vector.tensor_copy(v8_1[:, :, :, 65:MV], zpad[:])

            outT2 = qkvpool.tile([128, S], bf16, name="outT2")

            ident = qkvpool.tile([128, 128], f32, name="ident")
            make_identity(nc, ident[:])

            ctx_psav = tc.tile_pool(name="psav", bufs=2, space="PSUM")
            pavp = ctx_psav.__enter__()
            psavs = {}
            exp_state = {"emitted": 0, "av_done": 0, "pending": []}

            # ---- unit schedule ------------------------------------------
            # A unit is (sb, pair, t2): one head (pair), one 512-query strip
            # (sb), one 256-key chunk-pair (t2). AV accumulates over t2 in
            # order within each (sb, pair) group. The first two groups are
            # front-loaded during the QKV prologue (chunk availability grows
            # with the token quarters), at most 2 groups in flight (2 psav
            # PSUM buffers).
            units = []
            units += [(0, 0, t) for t in range(4)]                    # qtr 0
            units += [(0, 1, t) for t in range(4)]
            units += [(0, 0, t) for t in range(4, 8)]                 # qtr 1
            units += [(0, 1, t) for t in range(4, 8)]
            units += [(0, 0, t) for t in range(8, 12)]                # qtr 2
            units += [(0, 1, t) for t in range(8, 12)]
            units += [(0, 0, t) for t in range(12, 16)]               # main
            units += [(0, 1, t) for t in range(12, 16)]
            for sb in range(1, NSB):
                for pair in range(2):
                    units += [(sb, pair, t) for t in range(T2)]
            # scores+exp advance right after the quarter's K lands; AV
            # trails after the quarter's V (it needs v8, and the PE is
            # in-order so AV may not be emitted ahead of the transposes).
            N_EARLY_SC = [8, 16, 24]
            N_EARLY_AV = [6, 14, 22]

            def is_dve_unit(i):
                if i < 20:
                    return False  # prologue units: DVE is busy with QKV
                return units[i][2] in DVE_T2

            def emit_exp(pssc_ap, expT, dve):
                if dve:
                    nc.vector.tensor_scalar(
                        expT[:], pssc_ap, LOG2E, MAGIC_C,
                        mybir.AluOpType.mult, mybir.AluOpType.add,
                    )
                else:
                    nc.scalar.activation(expT[:].bitcast(f8), pssc_ap,
                                         AF.Exp, scale=0.125)

            def emit_av(u, expT):
                sb, pair, t2 = u
                v8 = v8_0 if pair == 0 else v8_1
                if t2 == 0:
                    psavs[(sb, pair)] = pavp.tile([MV, SBLK], f32,
                                                  name="psav")
                psav = psavs[(sb, pair)]
                return nc.tensor.matmul(
                    psav[:],
                    v8[:, t2, :, :],
                    expT[:].bitcast(f8),
                    start=(t2 == 0), stop=(t2 == T2 - 1),
                    perf_mode=DR,
                )

            # The normalization chain (denominator reciprocal + broadcast +
            # multiply) crosses engines five times. DVE reciprocal costs
            # ~7.5 cycles/element/lane, so the [1, 512] denominator row
            # would take ~4us on one lane (and reciprocal_approx_fast
            # returns garbage on real hw): spread the row across 128
            # partitions with a pair of tiny SBUF<->SBUF DMAs and run the
            # reciprocal at [128, 4]. The three DVE steps are emitted a few
            # units apart (phases) so the in-order DVE never head-of-line
            # blocks on a DMA that is still in flight.
            def norm_ph1(st):
                st["den"] = normpool.tile([1, SBLK], f32, name="den")
                nc.vector.tensor_copy(st["den"][:], st["psav"][64:65, :])
                st["dsp"] = normpool.tile([128, SBLK // 128], f32, name="dsp")
                nc.sync.dma_start(out=st["dsp"][:], in_=st["den"][:])

            def norm_ph2(st):
                st["rsp"] = normpool.tile([128, SBLK // 128], f32, name="rsp")
                nc.vector.reciprocal(st["rsp"][:], st["dsp"][:])
                st["recip"] = normpool.tile([1, SBLK], f32, name="recip")
                nc.sync.dma_start(out=st["recip"][:], in_=st["rsp"][:])
                st["rb"] = normpool.tile([64, SBLK], f32, name="rb")
                nc.gpsimd.partition_broadcast(st["rb"][:], st["recip"][:])

            def norm_ph3(st):
                sb, pair = st["sb"], st["pair"]
                p0 = pair * 64
                nc.vector.tensor_mul(
                    outT2[p0:p0 + 64, sb * SBLK:(sb + 1) * SBLK],
                    st["psav"][0:64, :], st["rb"][:],
                )

            NORM_PHASES = (norm_ph1, norm_ph2, norm_ph3)

            def emit_norm(u):
                # immediate full chain (used only at the drain tail)
                sb, pair, t2 = u
                st = {"sb": sb, "pair": pair, "psav": psavs.pop((sb, pair))}
                for ph in NORM_PHASES:
                    ph(st)

            # ---- fused Q/K/V prologue, quarter-major so compute chases
            # the x DMA. V is computed transposed (efficient N=512 matmuls)
            # and flipped into [t, d] fp8 layout with PE transposes.
            with (
                tc.tile_pool(name="psqk", bufs=3, space="PSUM") as pqkp,
                tc.tile_pool(name="pssce", bufs=2, space="PSUM") as pscep,
                tc.tile_pool(name="vt", bufs=2) as vtpool,
            ):
                def early_scores(u):
                    sb, pair, t2 = u
                    kTp = kTp0 if pair == 0 else kTp1
                    s0 = sb * SBLK
                    expT = exppool.tile([128, 2, SBLK], u8, name="expT")
                    for j in range(2):
                        t0 = (2 * t2 + j) * 128
                        pssc = pscep.tile([128, SBLK], f32, name="pssce")
                        nc.tensor.matmul(
                            pssc[:], kTp[:, t0:t0 + 128],
                            qT2[:, s0:s0 + SBLK],
                            start=True, stop=True,
                        )
                        nc.scalar.activation(expT[:, j, :].bitcast(f8),
                                             pssc[:], AF.Exp, scale=0.125)
                    return expT

                def early_scores_advance(k):
                    st = exp_state
                    while st["emitted"] < k:
                        i = st["emitted"]
                        st["pending"].append((units[i], early_scores(units[i])))
                        st["emitted"] = i + 1

                def early_av_advance(k):
                    st = exp_state
                    while st["av_done"] < k:
                        u, expT = st["pending"].pop(0)
                        emit_av(u, expT)
                        st["av_done"] += 1

                for qtr in range(4):
                    for which in range(3):
                        w_sb = (wq_sb, wk_sb, wv_sb)[which]
                        for scq in range(2):
                            sc = qtr * 2 + scq
                            s0 = sc * 512
                            psqk = pqkp.tile([128, 512], f32, name="psqk")
                            for cc in range(CC):
                                nc.tensor.matmul(
                                    psqk[:],
                                    w_sb[:, cc, :],
                                    tok_sb[:, cc, s0:s0 + 512],
                                    start=(cc == 0), stop=(cc == CC - 1),
                                )
                            if which == 0:
                                nc.vector.tensor_scalar_add(
                                    qT2[:, s0:s0 + 512], psqk[:], bq_sb[:, 0:1]
                                )
                            elif which == 1:
                                # no k bias: its score terms are query-only
                                # and cancel in softmax
                                nc.vector.tensor_copy(
                                    kTp0[0:64, s0:s0 + 512], psqk[0:64, :]
                                )
                                nc.vector.tensor_copy(
                                    kTp1[64:128, s0:s0 + 512], psqk[64:128, :]
                                )
                            else:
                                vt = vtpool.tile([128, 512], f32r, name="vt")
                                nc.vector.tensor_copy(vt[:], psqk[:])
                                # 4 transposes into one PSUM tile, then two
                                # strided fp8 copies peel the head halves.
                                pst4 = pqkp.tile([128, 512], f32, name="psqk")
                                for tt in range(4):
                                    nc.tensor.transpose(
                                        pst4[:, tt * 128:(tt + 1) * 128],
                                        vt[:, tt * 128:(tt + 1) * 128]
                                        .bitcast(f32),
                                        ident[:],
                                    )
                                # chunks sc*4 .. sc*4+3 -> t2 = sc*2, sc*2+1
                                t2a = sc * 2
                                src0 = pst4[:].rearrange(
                                    "p (c d) -> p c d", c=4)[:, :, 0:64]
                                src1 = pst4[:].rearrange(
                                    "p (c d) -> p c d", c=4)[:, :, 64:128]
                                dst0 = v8_0[:, t2a:t2a + 2, :, 0:64]
                                dst1 = v8_1[:, t2a:t2a + 2, :, 0:64]
                                nc.vector.tensor_copy(dst0, src0)
                                nc.vector.tensor_copy(dst1, src1)
                        if qtr < 3 and which == 1:
                            early_scores_advance(N_EARLY_SC[qtr])
                    if qtr < 3:
                        early_av_advance(N_EARLY_AV[qtr])

            # ---- attention stream + interleaved projection ---------------
            with (
                tc.tile_pool(name="pssc", bufs=3, space="PSUM") as pscp,
            ):
                pending_proj = []
                last_av = [None]

                def emit_scores(u):
                    sb, pair, t2 = u
                    kTp = kTp0 if pair == 0 else kTp1
                    s0 = sb * SBLK
                    pssc = pscp.tile([128, 2, SBLK], f32, name="pssc")
                    for j in range(2):
                        t0 = (2 * t2 + j) * 128
                        nc.tensor.matmul(
                            pssc[:, j, :], kTp[:, t0:t0 + 128],
                            qT2[:, s0:s0 + SBLK],
                            start=True, stop=True,
                        )
                    return pssc

                def emit_proj(sb, gate, half):
                    # proj borrows a pssc-pool tile (its two 512-wide halves
                    # hold two m-chunks) so the scores pipeline can be 3 deep
                    # within the 8 PSUM banks.
                    s0 = sb * SBLK
                    pspr = pscp.tile([128, 2, SBLK], f32, name="pssc")
                    for mh in range(2):
                        m = half * 2 + mh
                        mm = nc.tensor.matmul(
                            pspr[:, mh, :], wp_sb[:, m, :],
                            outT2[:, s0:s0 + SBLK],
                            start=True, stop=True,
                        )
                        if gate is not None:
                            # Keep proj behind the attention stream so the
                            # norm chain (recip etc.) finishes off-PE first.
                            add_dep_helper(mm.ins, gate.ins, sync=False,
                                           reason="defer proj past boundary")
                        po = outpool.tile([128, SBLK], f32, name="po")
                        # on ACT (Copy is in every table set): DVE is the
                        # busier engine in the steady state
                        nc.scalar.copy(po[:], pspr[:, mh, :])
                        nc.sync.dma_start(
                            out=o.ap()[m * 128:(m + 1) * 128, s0:s0 + SBLK],
                            in_=po[:],
                        )

                norm_q = []  # (due_step, phase_idx, state)
                step = [0]

                def run_due_norms():
                    while norm_q and norm_q[0][0] <= step[0]:
                        _, ph, st = norm_q.pop(0)
                        NORM_PHASES[ph](st)

                def av_and_norm(pu, pexp):
                    last_av[0] = emit_av(pu, pexp)
                    if pu[2] == T2 - 1:
                        sb, pair, _ = pu
                        st = {"sb": sb, "pair": pair,
                              "psav": psavs.pop((sb, pair))}
                        norm_q.extend([(step[0] + 1, 0, st),
                                       (step[0] + 3, 1, st),
                                       (step[0] + 5, 2, st)])
                        if pair == 1:
                            pending_proj.append(sb)

                # AV consumption lags the scores/exp stream by AV_LAG units
                # so the in-order PE never stalls waiting for an exp that
                # just issued (exp latency + sem delay ~1.3us would otherwise
                # gate every unit).
                start_i = exp_state["emitted"]
                pending = exp_state["pending"]
                for i in range(start_i, len(units)):
                    u = units[i]
                    pssc = emit_scores(u)
                    cur = exppool.tile([128, 2, SBLK], u8, name="expT")
                    emit_exp(pssc[:], cur, is_dve_unit(i))
                    pending.append((u, cur))
                    step[0] += 1
                    run_due_norms()
                    if len(pending) > AV_LAG:
                        av_and_norm(*pending.pop(0))
                    if pending_proj and (i % T2) == 10:
                        emit_proj(pending_proj[0], last_av[0], 0)
                    elif pending_proj and (i % T2) == 14:
                        emit_proj(pending_proj.pop(0), last_av[0], 1)
                for pu, pexp in pending:
                    step[0] += 1
                    run_due_norms()
                    av_and_norm(pu, pexp)
                step[0] += 99
                run_due_norms()
                for sb in pending_proj:
                    emit_proj(sb, None, 0)
                    emit_proj(sb, None, 1)
            ctx_psav.__exit__(None, None, None)

    nc.compile()
    return nc


def _prep_core_inputs(c, x, Wq, bq, Wk, bk, Wv, bv, Wp, bp):
    import ml_dtypes

    b = c // 4
    hs = 128 * (c % 4)
    bft = ml_dtypes.bfloat16

    def wslice_T(W):
        # W[hs:hs+128, :].T rearranged to [p, cc, d]
        return np.ascontiguousarray(
            W[hs:hs + 128, :].T.reshape(CC, 128, 128).transpose(1, 0, 2)
        ).astype(bft)

    wp_arr = np.ascontiguousarray(
        Wp[:, hs:hs + 128].reshape(CC, 128, 128).transpose(2, 0, 1)
    ).astype(bft)

    return {
        "xb": np.ascontiguousarray(x[b].reshape(C, S)).astype(bft),
        "wq": wslice_T(Wq),
        "wk": wslice_T(Wk),
        "wv": wslice_T(Wv),
        "wp": wp_arr,
        "bq": np.ascontiguousarray(bq[hs:hs + 128, None]).astype(np.float32),
    }


def kernel(x, Wq, bq, Wk, bk, Wv, bv, Wp, bp):
    global LAST_EXEC_NS, LAST_RESULTS
    from concourse.bass_utils import run_bass_kernel_spmd

    x, Wq, bq, Wk, bk, Wv, bv, Wp, bp = (
        np.asarray(a, dtype=np.float32)
        for a in (x, Wq, bq, Wk, bk, Wv, bv, Wp, bp)
    )

    if "nc" not in _cached:
        _cached["nc"] = _build()
    nc = _cached["nc"]

    in_maps = [
        _prep_core_inputs(c, x, Wq, bq, Wk, bk, Wv, bv, Wp, bp)
        for c in range(NCORES)
    ]
    trace = bool(os.environ.get("BASS_TRACE"))
    res = run_bass_kernel_spmd(nc, in_maps, core_ids=list(range(NCORES)),
                               trace=trace)
    LAST_RESULTS = res
    LAST_EXEC_NS = res.exec_time_ns

    # The projection bias (bp) and V's bias routed through the projection
    # (bv @ Wp^T) are constant per output channel: added host-side during
    # the partial-sum gather.
    bias_total = (bv.astype(np.float64) @ Wp.T.astype(np.float64)
                  + bp.astype(np.float64)).astype(np.float32)
    out = np.zeros((B, C, S), dtype=np.float32)
    for c in range(NCORES):
        out[c // 4] += res.results[c]["o"]
    out += bias_total[None, :, None]
    return out.reshape(B, C, HH, WW)
